# revision 1
# baseline (speedup 1.0000x reference)
"""Trainium2 Bass kernel for nn_ApproachPointPredictor (PointNet++-style FP decoder).

Sharding: data-parallel over batch B=32 -> 8 cores x 4 point clouds (weights
replicated). Per-core, per-cloud pipeline:
  fp3: k=1 interp from a single source point == broadcast of x3, so layer0
       splits into a per-cloud vector (x3 @ W0a) + per-point matmul (x2 @ W0b).
  fp2/fp1: exact kNN (k=3) via PE distance matmul (compensated bf16 hi/lo
       split, K=14, f32 PSUM accumulation -> ~2^-16 relative error on -d^2),
       DVE max8/max_index for top-4, inverse-d^2 weights, gpsimd local_scatter
       builds the weighted one-hot row, PE transposes it, dense matmul gathers
       and weight-sums the source features in one pass. MLPs with BN folded
       into weights on the host; head is Linear-ReLU-Linear-Sigmoid.
"""
import numpy as np

import concourse.bass as bass
import concourse.mybir as mybir
from concourse import tile
from concourse.bass_utils import run_bass_kernel_spmd

BF16 = mybir.dt.bfloat16
F32 = mybir.dt.float32
I16 = mybir.dt.int16
U32 = mybir.dt.uint32

NB = 4          # batches per core
N0, N1, N2, G = 4096, 1024, 256, 1024
BN_EPS = 1e-5


def build_core(nc: bass.Bass):
    def din(name, shape, dtype=F32):
        return nc.dram_tensor(name, shape, dtype, kind="ExternalInput")

    aq1 = din("aq1", [NB, 14, N1], BF16)
    aq0 = din("aq0", [NB, 14, N0], BF16)
    rs2 = din("rs2", [NB, 14, N2], BF16)
    rs1 = din("rs1", [NB, 14, N1], BF16)
    p1n = din("p1n", [NB, N1, 3])
    p0n = din("p0n", [NB, N0, 3])
    x3T = din("x3T", [G, NB], BF16)
    x2T = din("x2T", [NB, 256, N2], BF16)
    x1T = din("x1T", [NB, 128, N1], BF16)
    x0T = din("x0T", [NB, 3, N0], BF16)
    w3aT = din("w3aT", [G, 256], BF16)
    w3bT = din("w3bT", [256, 256], BF16)
    w3cT = din("w3cT", [256, 256], BF16)
    b3a = din("b3a", [128, 2])
    b3c = din("b3c", [128, 2])
    w2aT = din("w2aT", [256, 256], BF16)
    w2bT = din("w2bT", [128, 256], BF16)
    w2cT = din("w2cT", [256, 128], BF16)
    b2a = din("b2a", [128, 2])
    b2c = din("b2c", [128, 1])
    w1aT = din("w1aT", [128, 128], BF16)
    w1bT = din("w1bT", [3, 128], BF16)
    w1cT = din("w1cT", [128, 128], BF16)
    w1dT = din("w1dT", [128, 128], BF16)
    b1a = din("b1a", [128, 1])
    b1c = din("b1c", [128, 1])
    b1d = din("b1d", [128, 1])
    whaT = din("whaT", [128, 64], BF16)
    whbT = din("whbT", [64, 1], BF16)
    bha = din("bha", [64, 1])
    bhb = din("bhb", [1, 1])
    idnb = din("idnb", [128, 128], BF16)
    idnf = din("idnf", [4, 4])

    out = nc.dram_tensor("out", [NB, N0], F32, kind="ExternalOutput")

    ACT = mybir.ActivationFunctionType
    ALU = mybir.AluOpType
    AX = mybir.AxisListType

    from contextlib import ExitStack
    with tile.TileContext(nc) as tc, ExitStack() as ctx:
        cpool = ctx.enter_context(tc.tile_pool(name="const", bufs=1))
        sb = ctx.enter_context(tc.tile_pool(name="sb", bufs=2))
        sb3 = ctx.enter_context(tc.tile_pool(name="sb3", bufs=2))
        big1 = ctx.enter_context(tc.tile_pool(name="big1", bufs=1))
        pers = ctx.enter_context(tc.tile_pool(name="pers", bufs=1))
        ps_nd = ctx.enter_context(tc.tile_pool(name="ps_nd", bufs=2, space="PSUM"))
        ps_tp = ctx.enter_context(tc.tile_pool(name="ps_tp", bufs=1, space="PSUM"))
        ps_mm = ctx.enter_context(tc.tile_pool(name="ps_mm", bufs=2, space="PSUM"))

        def ldconst(t, dtype=None):
            shape = list(t.shape)
            ap = t[:]
            if shape[0] > 128:
                k = shape[0] // 128
                ap = ap.rearrange("(k p) ... -> p k ...", p=128)
                shape = [128, k] + shape[1:]
            s = cpool.tile(shape, dtype or t.dtype, tag=t.name)
            nc.sync.dma_start(s[:], ap)
            return s

        idnb_s = ldconst(idnb)
        idnf_s = ldconst(idnf)
        w3a_s = ldconst(w3aT)
        w3b_s = ldconst(w3bT)
        w3c_s = ldconst(w3cT)
        b3a_s = ldconst(b3a)
        b3c_s = ldconst(b3c)
        w2a_s = ldconst(w2aT)
        w2b_s = ldconst(w2bT)
        w2c_s = ldconst(w2cT)
        b2a_s = ldconst(b2a)
        b2c_s = ldconst(b2c)
        w1a_s = ldconst(w1aT)
        w1b_s = ldconst(w1bT)
        w1c_s = ldconst(w1cT)
        w1d_s = ldconst(w1dT)
        b1a_s = ldconst(b1a)
        b1c_s = ldconst(b1c)
        b1d_s = ldconst(b1d)
        wha_s = ldconst(whaT)
        whb_s = ldconst(whbT)
        bha_s = ldconst(bha)
        bhb_s = ldconst(bhb)
        x3T_s = ldconst(x3T)

        ps_u = ps_mm.tile([NB, 256], F32, tag="mlp")
        for kt in range(8):
            nc.tensor.matmul(ps_u[:], x3T_s[:, kt, :], w3a_s[:, kt, :],
                             start=(kt == 0), stop=(kt == 7))
        u_sb = pers.tile([NB, 256], F32, tag="u_sb")
        nc.scalar.activation(u_sb[:], ps_u[:], ACT.Copy)
        bias3 = pers.tile([128, 2, NB], F32, tag="bias3")
        for ct in range(2):
            pt = ps_tp.tile([128, NB], F32, tag="ip")
            nc.tensor.transpose(pt[:], u_sb[:, bass.ts(ct, 128)], idnf_s[:])
            nc.vector.tensor_tensor(bias3[:, ct, :], pt[:],
                                    b3a_s[:, ct][:, None].broadcast_to((128, NB)),
                                    op=ALU.add)

        feat2N = pers.tile([128, 2, 2, 128], BF16, tag="feat2N")
        feat1N = pers.tile([128, 8, 128], BF16, tag="feat1N")
        interp2 = pers.tile([128, 2, N1], BF16, tag="interp2")
        interp1 = pers.tile([128, N0], BF16, tag="interp1")

        def knn_interp(aq, qn, Nq, rsrc, Ns, featN_mm):
            rhsD = big1.tile([14, Ns], BF16, tag="rhsD")
            nc.sync.dma_start(rhsD[:], rsrc[:])
            augQ = big1.tile([14, Nq], BF16, tag="augQ")
            nc.sync.dma_start(augQ[:], aq[:])

            nqt = Nq // 128
            nst = Ns // 128
            for qt in range(nqt):
                sqq = sb3.tile([128, 3], F32, tag="sqq")
                nc.scalar.activation(sqq[:], qn[:, qt, :], ACT.Square)
                q2 = sb3.tile([128, 1], F32, tag="q2")
                nc.vector.tensor_reduce(q2[:], sqq[:], axis=AX.X, op=ALU.add)
                nd = ps_nd.tile([128, Ns], F32, tag="nd")
                for j in range(max(1, Ns // 512)):
                    n0, n1x = j * 512, min(Ns, (j + 1) * 512)
                    nc.tensor.matmul(nd[:, n0:n1x], augQ[:, bass.ts(qt, 128)],
                                     rhsD[:, n0:n1x], start=True, stop=True)
                nd_sb = sb3.tile([128, Ns], F32, tag="nd_sb")
                nc.scalar.activation(nd_sb[:], nd[:], ACT.Copy)
                v8 = sb3.tile([128, 8], F32, tag="v8")
                nc.vector.max(v8[:], nd_sb[:])
                i8 = sb3.tile([128, 8], U32, tag="i8")
                nc.vector.max_index(i8[:], v8[:], nd_sb[:])
                d24 = sb3.tile([128, 4], F32, tag="d24")
                nc.vector.tensor_scalar(d24[:], v8[:, 0:4], -1.0, q2[:],
                                        op0=ALU.mult, op1=ALU.add)
                nc.vector.tensor_scalar_max(d24[:], d24[:], 1e-12)
                w4 = sb3.tile([128, 4], F32, tag="w4")
                nc.vector.reciprocal(w4[:], d24[:])
                nc.vector.memset(w4[:, 3:4], 0.0)
                sw = sb3.tile([128, 1], F32, tag="sw")
                nc.vector.tensor_reduce(sw[:], w4[:, 0:3], axis=AX.X, op=ALU.add)
                rsw = sb3.tile([128, 1], F32, tag="rsw")
                nc.vector.reciprocal(rsw[:], sw[:])
                a4 = sb3.tile([128, 4], BF16, tag="a4")
                nc.vector.tensor_scalar(a4[:], w4[:], rsw[:], None, op0=ALU.mult)
                i16 = sb3.tile([128, 4], I16, tag="i16")
                nc.vector.tensor_copy(i16[:], i8[:, 0:4])
                wm = sb3.tile([128, Ns], BF16, tag="wm")
                nc.gpsimd.local_scatter(wm[:], a4[:], i16[:], channels=128,
                                        num_elems=Ns, num_idxs=4)
                wmt_ps = ps_tp.tile([128, nst, 128], BF16, tag="tp_bf")
                for st in range(nst):
                    nc.tensor.transpose(wmt_ps[:, st, :], wm[:, bass.ts(st, 128)],
                                        idnb_s[:])
                wmt = sb3.tile([128, nst, 128], BF16, tag="wmt")
                if qt % 2 == 0:
                    nc.vector.tensor_copy(wmt[:], wmt_ps[:])
                else:
                    nc.scalar.activation(wmt[:], wmt_ps[:], ACT.Copy)
                featN_mm(qt, wmt)

        for b in range(NB):
            x2b = sb.tile([128, 2, N2], BF16, tag="x2b")
            nc.sync.dma_start(x2b[:], x2T[b].rearrange("(k p) n -> p k n", p=128))
            h2T = sb.tile([128, 2, N2], BF16, tag="h2T")
            for ct in range(2):
                pm = ps_mm.tile([128, N2], F32, tag="mlp")
                for kt in range(2):
                    nc.tensor.matmul(pm[:], w3b_s[:, kt, bass.ts(ct, 128)],
                                     x2b[:, kt, :], start=(kt == 0), stop=(kt == 1))
                nc.scalar.activation(h2T[:, ct, :], pm[:], ACT.Relu,
                                     bias=bias3[:, ct, b][:, None])
            f2T = sb.tile([128, 2, N2], BF16, tag="f2T")
            for ct in range(2):
                pm = ps_mm.tile([128, N2], F32, tag="mlp")
                for kt in range(2):
                    nc.tensor.matmul(pm[:], w3c_s[:, kt, bass.ts(ct, 128)],
                                     h2T[:, kt, :], start=(kt == 0), stop=(kt == 1))
                nc.scalar.activation(f2T[:, ct, :], pm[:], ACT.Identity,
                                     bias=b3c_s[:, ct][:, None])
            f2ps = ps_tp.tile([128, 2, 2, 128], BF16, tag="tp_bf")
            for st in range(2):
                for ct in range(2):
                    nc.tensor.transpose(f2ps[:, st, ct, :],
                                        f2T[:, ct, bass.ts(st, 128)], idnb_s[:])
            nc.vector.tensor_copy(feat2N[:], f2ps[:])

            p1nb = sb.tile([128, 8, 3], F32, tag="p1nb")
            nc.sync.dma_start(p1nb[:], p1n[b].rearrange("(t p) c -> p t c", p=128))

            def mm2(qt, wmt):
                for ct in range(2):
                    ip = ps_tp.tile([128, 128], F32, tag="ip")
                    for st in range(2):
                        nc.tensor.matmul(ip[:], feat2N[:, st, ct, :], wmt[:, st, :],
                                         start=(st == 0), stop=(st == 1))
                    if (qt + ct) % 2 == 0:
                        nc.vector.tensor_copy(interp2[:, ct, bass.ts(qt, 128)], ip[:])
                    else:
                        nc.scalar.activation(interp2[:, ct, bass.ts(qt, 128)], ip[:],
                                             ACT.Copy)

            knn_interp(aq1[b], p1nb, N1, rs2[b], N2, mm2)

            x1b = sb.tile([128, N1], BF16, tag="x1b")
            nc.sync.dma_start(x1b[:], x1T[b])
            h2m = sb.tile([128, 2, N1], BF16, tag="h2m")
            for ot in range(2):
                for j in range(2):
                    nsl = bass.ts(j, 512)
                    pm = ps_mm.tile([128, 512], F32, tag="mlp")
                    for kt in range(2):
                        nc.tensor.matmul(pm[:], w2a_s[:, kt, bass.ts(ot, 128)],
                                         interp2[:, kt, nsl],
                                         start=(kt == 0), stop=False)
                    nc.tensor.matmul(pm[:], w2b_s[:, bass.ts(ot, 128)], x1b[:, nsl],
                                     start=False, stop=True)
                    nc.scalar.activation(h2m[:, ot, nsl], pm[:], ACT.Relu,
                                         bias=b2a_s[:, ot][:, None])
            h1T = sb.tile([128, N1], BF16, tag="h1T")
            for j in range(2):
                nsl = bass.ts(j, 512)
                pm = ps_mm.tile([128, 512], F32, tag="mlp")
                for kt in range(2):
                    nc.tensor.matmul(pm[:], w2c_s[:, kt, :], h2m[:, kt, nsl],
                                     start=(kt == 0), stop=(kt == 1))
                nc.scalar.activation(h1T[:, nsl], pm[:], ACT.Identity, bias=b2c_s[:])
            f1ps = ps_tp.tile([128, 8, 128], BF16, tag="tp_bf")
            for st in range(8):
                nc.tensor.transpose(f1ps[:, st, :], h1T[:, bass.ts(st, 128)], idnb_s[:])
            nc.scalar.activation(feat1N[:], f1ps[:], ACT.Copy)

            p0nb = sb.tile([128, 32, 3], F32, tag="p0nb")
            nc.sync.dma_start(p0nb[:], p0n[b].rearrange("(t p) c -> p t c", p=128))

            def mm1(qt, wmt):
                ip = ps_tp.tile([128, 128], F32, tag="ip")
                for st in range(8):
                    nc.tensor.matmul(ip[:], feat1N[:, st, :], wmt[:, st, :],
                                     start=(st == 0), stop=(st == 7))
                if qt % 2 == 0:
                    nc.scalar.activation(interp1[:, bass.ts(qt, 128)], ip[:], ACT.Copy)
                else:
                    nc.vector.tensor_copy(interp1[:, bass.ts(qt, 128)], ip[:])

            knn_interp(aq0[b], p0nb, N0, rs1[b], N1, mm1)

            x0b = big1.tile([3, N0], BF16, tag="x0b")
            nc.sync.dma_start(x0b[:], x0T[b])
            g1 = big1.tile([128, N0], BF16, tag="g1")
            for j in range(8):
                nsl = bass.ts(j, 512)
                pm = ps_mm.tile([128, 512], F32, tag="mlp")
                nc.tensor.matmul(pm[:], w1a_s[:], interp1[:, nsl],
                                 start=True, stop=False)
                nc.tensor.matmul(pm[:], w1b_s[:], x0b[:, nsl],
                                 start=False, stop=True)
                nc.scalar.activation(g1[:, nsl], pm[:], ACT.Relu, bias=b1a_s[:])
            g2 = big1.tile([128, N0], BF16, tag="g2")
            for j in range(8):
                nsl = bass.ts(j, 512)
                pm = ps_mm.tile([128, 512], F32, tag="mlp")
                nc.tensor.matmul(pm[:], w1c_s[:], g1[:, nsl], start=True, stop=True)
                nc.scalar.activation(g2[:, nsl], pm[:], ACT.Relu, bias=b1c_s[:])
            g3 = big1.tile([128, N0], BF16, tag="g3")
            for j in range(8):
                nsl = bass.ts(j, 512)
                pm = ps_mm.tile([128, 512], F32, tag="mlp")
                nc.tensor.matmul(pm[:], w1d_s[:], g2[:, nsl], start=True, stop=True)
                nc.scalar.activation(g3[:, nsl], pm[:], ACT.Identity, bias=b1d_s[:])
            hh = big1.tile([64, N0], BF16, tag="hh")
            for j in range(8):
                nsl = bass.ts(j, 512)
                pm = ps_mm.tile([64, 512], F32, tag="mlp")
                nc.tensor.matmul(pm[:], wha_s[:], g3[:, nsl], start=True, stop=True)
                nc.scalar.activation(hh[:, nsl], pm[:], ACT.Relu, bias=bha_s[:])
            ob = sb.tile([1, N0], F32, tag="ob")
            for j in range(8):
                nsl = bass.ts(j, 512)
                pm = ps_mm.tile([1, 512], F32, tag="mlp")
                nc.tensor.matmul(pm[:], whb_s[:], hh[:, nsl], start=True, stop=True)
                nc.scalar.activation(ob[:, nsl], pm[:], ACT.Sigmoid, bias=bhb_s[:])
            nc.sync.dma_start(out[b, :][None, :], ob[:])

    return nc


def _fold(W, b, g, beta):
    s = np.asarray(g) / np.sqrt(1.0 + BN_EPS)
    return (np.asarray(W) * s[:, None]).astype(np.float32), \
        (np.asarray(b) * s + np.asarray(beta)).astype(np.float32)


def _split_pos(pos, qform):
    import ml_dtypes
    bf16 = ml_dtypes.bfloat16
    t = np.transpose(np.asarray(pos, np.float32), (0, 2, 1))
    h = t.astype(bf16).astype(np.float32)
    l = t - h
    nb, _, N = t.shape
    outm = np.zeros((nb, 14, N), np.float32)
    if qform:   # rows pair with source rows [-s2h, -s2l, sh, 2sh, sl, 2sl]
        outm[:, 0:2] = 1.0
        outm[:, 2:5] = 2.0 * h
        outm[:, 5:8] = l
        outm[:, 8:11] = 2.0 * h
        outm[:, 11:14] = l
    else:
        s2 = np.sum(t * t, axis=1, keepdims=True)
        s2h = s2.astype(bf16).astype(np.float32)
        outm[:, 0:1] = -s2h
        outm[:, 1:2] = -(s2 - s2h)
        outm[:, 2:5] = h
        outm[:, 5:8] = 2.0 * h
        outm[:, 8:11] = l
        outm[:, 11:14] = 2.0 * l
    return outm.astype(bf16)


def kernel(**inp):
    import ml_dtypes
    bf16 = ml_dtypes.bfloat16
    f32 = np.float32

    w3, bb3 = _fold(inp["fp3_W0"], inp["fp3_b0"], inp["fp3_g0"], inp["fp3_beta0"])
    w2, bb2 = _fold(inp["fp2_W0"], inp["fp2_b0"], inp["fp2_g0"], inp["fp2_beta0"])
    w1, bb1 = _fold(inp["fp1_W0"], inp["fp1_b0"], inp["fp1_g0"], inp["fp1_beta0"])
    w1c, bb1c = _fold(inp["fp1_W1"], inp["fp1_b1"], inp["fp1_g1"], inp["fp1_beta1"])

    def bl(v, nt):
        return np.ascontiguousarray(np.asarray(v, f32).reshape(nt, 128).T)


    shared = {
        "w3aT": np.ascontiguousarray(w3[:, :G].T).astype(bf16),
        "w3bT": np.ascontiguousarray(w3[:, G:].T).astype(bf16),
        "w3cT": np.ascontiguousarray(np.asarray(inp["fp3_W1"]).T).astype(bf16),
        "b3a": bl(bb3, 2), "b3c": bl(inp["fp3_b1"], 2),
        "w2aT": np.ascontiguousarray(w2[:, :256].T).astype(bf16),
        "w2bT": np.ascontiguousarray(w2[:, 256:].T).astype(bf16),
        "w2cT": np.ascontiguousarray(np.asarray(inp["fp2_W1"]).T).astype(bf16),
        "b2a": bl(bb2, 2), "b2c": bl(inp["fp2_b1"], 1),
        "w1aT": np.ascontiguousarray(w1[:, :128].T).astype(bf16),
        "w1bT": np.ascontiguousarray(w1[:, 128:].T).astype(bf16),
        "w1cT": np.ascontiguousarray(w1c.T).astype(bf16),
        "w1dT": np.ascontiguousarray(np.asarray(inp["fp1_W2"]).T).astype(bf16),
        "b1a": bl(bb1, 1), "b1c": bl(bb1c, 1), "b1d": bl(inp["fp1_b2"], 1),
        "whaT": np.ascontiguousarray(np.asarray(inp["head_W0"]).T).astype(bf16),
        "whbT": np.ascontiguousarray(np.asarray(inp["head_W1"]).T).astype(bf16),
        "bha": np.asarray(inp["head_b0"], f32).reshape(64, 1),
        "bhb": np.asarray(inp["head_b1"], f32).reshape(1, 1),
        "idnb": np.eye(128, dtype=bf16),
        "idnf": np.eye(4, dtype=f32),
    }

    def tr(p):
        return np.ascontiguousarray(np.transpose(np.asarray(p, f32), (0, 2, 1)))

    in_maps = []
    for c in range(8):
        s = slice(c * NB, (c + 1) * NB)
        m = dict(shared)
        m["aq1"] = _split_pos(inp["pos1"][s], True)
        m["aq0"] = _split_pos(inp["pos0"][s], True)
        m["rs2"] = _split_pos(inp["pos2"][s], False)
        m["rs1"] = _split_pos(inp["pos1"][s], False)
        m["p1n"] = np.asarray(inp["pos1"][s], f32)
        m["p0n"] = np.asarray(inp["pos0"][s], f32)
        m["x3T"] = np.ascontiguousarray(np.asarray(inp["x3"])[s, 0, :].T).astype(bf16)
        m["x2T"] = tr(inp["x2"][s]).astype(bf16)
        m["x1T"] = tr(inp["x1"][s]).astype(bf16)
        m["x0T"] = tr(inp["x0"][s]).astype(bf16)
        in_maps.append(m)

    from concourse.bacc import Bacc
    nc = Bacc()
    build_core(nc)
    nc.finalize()

    res = run_bass_kernel_spmd(nc, in_maps, core_ids=list(range(8)))
    outs = [r["out"] if isinstance(r, dict) else r for r in res.results]
    full = np.concatenate([np.asarray(o, np.float32).reshape(NB, N0, 1) for o in outs],
                          axis=0)
    return full

